# revision 8
# baseline (speedup 1.0000x reference)
"""Causal self-attention (B=2, T=2048, C=1024, H=16) on 8 trn2 NeuronCores.

Sharding: core c handles batch b = c // 4 and head-group g = c % 4 (4 heads).
Each core computes its heads' attention and a partial output projection
(rows 256g:256g+256 of w_proj); the host sums the 4 partials per batch and
adds b_proj (+ the v-bias fold b_v @ w_proj).

v2 layout: host supplies x pre-transposed in bf16 (xT [C, T]); QKV runs as
one GEMM loop producing qT/kT/vT pair-packed tiles; v is re-transposed on
device into the PV stationary layout with an appended ones column that
yields softmax denominators for free.  B(tb) -> C(tb) are software-pipelined
per 512-wide t-block so the scalar-engine exp hides under tensor work.

Self-contained: hardcodes all shapes; only needs concourse (on sys.path via
the environment) and numpy.
"""
import numpy as np

import concourse.bacc as bacc
import concourse.tile as tile
import concourse.mybir as mybir
from concourse.bass_utils import run_bass_kernel_spmd

F32 = mybir.dt.float32
BF16 = mybir.dt.bfloat16

B, T, C = 2, 2048, 1024
N_HEAD = 16
D = C // N_HEAD          # 64
SCALE = D ** -0.5
HL = 4                   # heads per core (local)
CL = HL * D              # 256 local feature cols per q/k/v section
TB = T // 512            # 4 t-blocks of 512
TT = T // 128            # 16 t-tiles of 128
CT = C // 128            # 8 contraction tiles
VW = D + 1               # 65: v columns per head (+ ones col for denominator)
EXP = mybir.ActivationFunctionType.Exp


def _build(dbg=False):
    nc = bacc.Bacc("TRN2", debug=False)
    xT_d = nc.dram_tensor("xT", [C, T], BF16, kind="ExternalInput").ap()
    w_d = nc.dram_tensor("w", [C, 3 * CL], BF16, kind="ExternalInput").ap()
    b_d = nc.dram_tensor("b", [128, 4], F32, kind="ExternalInput").ap()
    wp_d = nc.dram_tensor("wp", [2 * 128, C], BF16, kind="ExternalInput").ap()
    tri2_d = nc.dram_tensor("tri2", [128, 256], BF16, kind="ExternalInput").ap()
    out_d = nc.dram_tensor("out", [T, C], BF16, kind="ExternalOutput").ap()
    if dbg:
        dqk_d = nc.dram_tensor("dqk", [128, 4 * T], BF16, kind="ExternalOutput").ap()
        dva_d = nc.dram_tensor("dva", [128, TT * 4 * VW], BF16, kind="ExternalOutput").ap()
        dyt_d = nc.dram_tensor("dyt", [128, 2 * T], BF16, kind="ExternalOutput").ap()
        dbc_d = nc.dram_tensor("dbc", [64, 8 * 512], F32, kind="ExternalOutput").ap()

    with tile.TileContext(nc) as tc:
        with tc.tile_pool(name="persist", bufs=1) as pers:
            w6 = pers.tile([128, CT, 3 * CL], BF16)
            xT = pers.tile([128, CT, T], BF16)
            w_r = w_d.rearrange("(g p) j -> p g j", p=128)
            xT_r = xT_d.rearrange("(g p) t -> p g t", p=128)
            # per-g interleaved loads so B(0) can start after ~2 small DMAs
            for g in range(CT):
                nc.sync.dma_start(out=w6[:, g:g + 1, :], in_=w_r[:, g:g + 1, :])
                nc.sync.dma_start(
                    out=xT[:, g:g + 1, 0:512], in_=xT_r[:, g:g + 1, 0:512])
            tri2 = pers.tile([128, 256], BF16)
            nc.sync.dma_start(out=tri2, in_=tri2_d)
            b6 = pers.tile([128, 4], F32)
            nc.sync.dma_start(out=b6, in_=b_d)
            wp = pers.tile([128, 2, C], BF16)
            nc.sync.dma_start(out=wp, in_=wp_d.rearrange("(g p) j -> p g j", p=128))
            for tb in range(1, TB):
                cs = slice(tb * 512, (tb + 1) * 512)
                nc.sync.dma_start(out=xT[:, :, cs], in_=xT_r[:, :, cs])

            # persistent intermediates
            qkT = pers.tile([128, 4, T], BF16)       # q01 | q23 | k01 | k23
            vall = pers.tile([128, TT, 2 * 2 * VW], BF16)
            yT = pers.tile([128, 2, T], BF16)        # per pair: [dA(64)|dB(64)] x t

            # ones columns of v_aug (denominator trick)
            vones = vall.rearrange("p s (h w) -> p s h w", w=VW)[:, :, :, D:VW]
            nc.gpsimd.memset(vones, 1.0)

            with tc.tile_pool(name="sbB", bufs=1) as sbb, \
                 tc.tile_pool(name="psBC", bufs=1, space="PSUM") as psp:
                for tb in range(TB):
                    tsl = slice(tb * 512, (tb + 1) * 512)
                    # ---- B(tb): qkv projections for this t-block ----
                    # jt: 0,1 = q pairs; 2,3 = k pairs; 4,5 = v pairs
                    for jt in (2, 3, 0, 1):
                        ps = psp.tile([128, 512], F32, tag="qk", bufs=2)
                        for g in range(CT):
                            nc.tensor.matmul(
                                ps,
                                w6[:, g, jt * 128:(jt + 1) * 128],
                                xT[:, g, tsl],
                                start=(g == 0), stop=(g == CT - 1))
                        nc.vector.tensor_scalar_add(
                            qkT[:, jt, tsl], ps, b6[:, jt:jt + 1])
                    # v in natural [s, d] layout (PV stationary)
                    for tt in range(4):
                        si = 4 * tb + tt
                        ksl = slice(si * 128, (si + 1) * 128)
                        psv = psp.tile([128, 256], F32, tag="qk", bufs=2)
                        for g in range(CT):
                            nc.tensor.matmul(
                                psv,
                                xT[:, g, ksl],
                                w6[:, g, 2 * CL:3 * CL],
                                start=(g == 0), stop=(g == CT - 1))
                        dst = vall[:, si, :].rearrange(
                            "p (h w) -> p h w", w=VW)[:, :, 0:D]
                        src = psv.rearrange("p (h w) -> p h w", w=D)
                        nc.vector.tensor_copy(dst, src)

                    # ---- C(tb): attention for this t-block, both pairs ----
                    n_si = 4 * (tb + 1)
                    for p in range(2):
                        ypsA = psp.tile([VW, 512], F32, tag="ypsA", bufs=1)
                        ypsB = psp.tile([VW, 512], F32, tag="ypsB", bufs=1)
                        for si in range(n_si):
                            k = si - 4 * tb
                            col0 = 128 * k if k >= 0 else 0
                            st = psp.tile([128, 1024], F32, tag="st", bufs=2)
                            ssl = slice(si * 128, (si + 1) * 128)
                            qsl = slice(tb * 512 + col0, (tb + 1) * 512)
                            nc.tensor.matmul(
                                st[:, col0:512],
                                qkT[0:64, 2 + p, ssl], qkT[0:64, p, qsl],
                                tile_position=(0, 0), start=True, stop=True)
                            nc.tensor.matmul(
                                st[:, 512 + col0:1024],
                                qkT[64:128, 2 + p, ssl], qkT[64:128, p, qsl],
                                tile_position=(64, 0), start=True, stop=True)
                            pt = sbb.tile([128, 1024], BF16, tag="pt", bufs=3)
                            st3 = st.rearrange("p (h q) -> p h q", h=2)[:, :, col0:512]
                            pt3 = pt.rearrange("p (h q) -> p h q", h=2)[:, :, col0:512]
                            nc.scalar.activation(pt3, st3, EXP, scale=SCALE)
                            if k >= 0:
                                strip = pt.rearrange(
                                    "p (h q) -> p h q", h=2)[:, :, col0:col0 + 128]
                                nc.vector.tensor_mul(
                                    strip, strip,
                                    tri2.rearrange("p (h q) -> p h q", h=2))
                            voff = p * 2 * VW
                            nc.tensor.matmul(
                                ypsA[:, col0:512],
                                vall[:, si, voff:voff + VW],
                                pt[:, col0:512],
                                start=(si == 0), stop=(si == n_si - 1),
                                skip_group_check=True)
                            nc.tensor.matmul(
                                ypsB[:, col0:512],
                                vall[:, si, voff + VW:voff + 2 * VW],
                                pt[:, 512 + col0:1024],
                                start=(si == 0), stop=(si == n_si - 1),
                                skip_group_check=True)
                        den2 = sbb.tile([1, 2, 512], F32, tag="den2", bufs=2)
                        nc.vector.tensor_copy(den2[:, 0, :], ypsA[D:VW, :])
                        nc.vector.tensor_copy(den2[:, 1, :], ypsB[D:VW, :])
                        recipA = sbb.tile([1, 512], F32, tag="recipA", bufs=2)
                        recipB = sbb.tile([1, 512], F32, tag="recipB", bufs=2)
                        nc.vector.reciprocal_approx_fast(
                            out=recipA, in_=den2[:, 0, :])
                        nc.vector.reciprocal_approx_fast(
                            out=recipB, in_=den2[:, 1, :])
                        bcA = sbb.tile([64, 512], F32, tag="bcA", bufs=2)
                        bcB = sbb.tile([64, 512], F32, tag="bcB", bufs=2)
                        nc.gpsimd.partition_broadcast(bcA, recipA, channels=64)
                        nc.gpsimd.partition_broadcast(bcB, recipB, channels=64)
                        nc.vector.tensor_mul(yT[0:64, p, tsl], ypsA[0:D, :], bcA)
                        nc.vector.tensor_mul(yT[64:128, p, tsl], ypsB[0:D, :], bcB)
                        if dbg:
                            nc.sync.dma_start(
                                out=dbc_d[:, (4 * p + tb) * 512:(4 * p + tb + 1) * 512],
                                in_=bcA)

                    # ---- D(tb): output projection for this t-block ----
                    for ti in range(4 * tb, 4 * tb + 4):
                        o_sb = sbb.tile([128, C], BF16, tag="osb", bufs=3)
                        ksl = slice(ti * 128, (ti + 1) * 128)
                        for cb in range(2):
                            pp = psp.tile([128, 512], F32, tag="qk", bufs=2)
                            csl = slice(cb * 512, (cb + 1) * 512)
                            for p in range(2):
                                nc.tensor.matmul(
                                    pp, yT[:, p, ksl], wp[:, p, csl],
                                    start=(p == 0), stop=(p == 1))
                            dst = o_sb[:, csl]
                            if cb == 0:
                                nc.vector.tensor_copy(dst, pp)
                            else:
                                nc.scalar.copy(dst, pp)
                        nc.sync.dma_start(out=out_d[ksl, :], in_=o_sb)

            if dbg:
                nc.sync.dma_start(out=dqk_d, in_=qkT.rearrange("p a t -> p (a t)"))
                nc.sync.dma_start(out=dva_d, in_=vall.rearrange("p s w -> p (s w)"))
                nc.sync.dma_start(out=dyt_d, in_=yT.rearrange("p a t -> p (a t)"))

    nc.compile()
    return nc


_NC = None


def _get_nc():
    global _NC
    if _NC is None:
        _NC = _build()
    return _NC


def _make_in_maps(x, w_attn, b_attn, w_proj):
    import ml_dtypes
    tri2 = np.zeros((128, 256), dtype=np.float32)
    i = np.arange(128)[:, None]
    j = np.arange(128)[None, :]
    tri = (j >= i).astype(np.float32)
    tri2[:, 0:128] = tri
    tri2[:, 128:256] = tri
    in_maps = []
    for c in range(8):
        b = c // 4
        g = c % 4
        qs = slice(256 * g, 256 * g + 256)
        ks = slice(C + 256 * g, C + 256 * g + 256)
        vs = slice(2 * C + 256 * g, 2 * C + 256 * g + 256)
        w_local = np.concatenate(
            [w_attn[:, qs], w_attn[:, ks], w_attn[:, vs]], axis=1)
        b_local = np.concatenate([b_attn[qs], b_attn[ks]])
        b6 = np.ascontiguousarray(
            b_local.reshape(4, 128).T, dtype=np.float32)
        wp_local = w_proj[256 * g:256 * g + 256, :]
        xT_local = np.ascontiguousarray(x[b].T)
        in_maps.append({
            "xT": xT_local.astype(ml_dtypes.bfloat16),
            "w": np.ascontiguousarray(w_local).astype(ml_dtypes.bfloat16),
            "b": b6,
            "wp": np.ascontiguousarray(wp_local).astype(ml_dtypes.bfloat16),
            "tri2": tri2.astype(ml_dtypes.bfloat16),
        })
    return in_maps


def run(x, w_attn, b_attn, w_proj, b_proj, trace=False, tmpdir=None):
    x = np.asarray(x, dtype=np.float32)
    w_attn = np.asarray(w_attn, dtype=np.float32)
    b_attn = np.asarray(b_attn, dtype=np.float32)
    w_proj = np.asarray(w_proj, dtype=np.float32)
    b_proj = np.asarray(b_proj, dtype=np.float32)
    nc = _get_nc()
    in_maps = _make_in_maps(x, w_attn, b_attn, w_proj)
    res = run_bass_kernel_spmd(
        nc, in_maps, core_ids=list(range(8)), trace=trace, tmpdir=tmpdir)
    parts = [np.asarray(res.results[c]["out"], dtype=np.float32)
             for c in range(8)]
    out = np.empty((B, T, C), dtype=np.float32)
    for b in range(2):
        out[b] = parts[4 * b] + parts[4 * b + 1] + parts[4 * b + 2] + parts[4 * b + 3]
    # bias + v-bias fold: y uses v without bias; b_v @ w_proj restores it.
    bv = b_attn[2 * C:3 * C].astype(np.float32)
    out += (b_proj + bv @ w_proj)[None, None, :]
    return out, res


def kernel(x, w_attn, b_attn, w_proj, b_proj):
    out, _ = run(x, w_attn, b_attn, w_proj, b_proj, trace=False)
    return out
